# revision 3
# baseline (speedup 1.0000x reference)
"""MoE GroupedExperts kernel for 8 TRN2 NeuronCores.

Expert-parallel: expert e's tokens + weights go to core e. Tokens are
pre-sorted by expert, so routing is host-side slicing. Each core runs a
SwiGLU MLP: o = (silu(x @ gate) * (x @ up)) @ down.

Weights are repacked host-side into partition-major, chunk-major DRAM
layout so every weight DMA is 128 descriptors x 4KB contiguous runs
(vs 1024 x 256-512B in the natural [DIM, HID] layout) -- this moves
effective HBM bandwidth from ~268 GB/s to near the ~358 GB/s cap, which
is what paces this kernel. The PE is kept warm from t~7us (framework
preamble end) with throwaway matmuls so real GEMMs run at 2.4 GHz.
"""

import sys

if "/opt/trn_rl_repo" not in sys.path:
    sys.path.insert(0, "/opt/trn_rl_repo")

import numpy as np

BF16 = np.float16
E = 8
DIM = 1024
HID = 2048
N_CORES = 8
CMAX_BLOCK = 512  # max tokens per device invocation (PSUM free-dim limit)

_cache = {}


def _build(cpad: int):
    """Build + compile the per-core kernel for cpad tokens per expert."""
    from concourse import bacc
    import concourse.tile as tile
    import concourse.mybir as mybir

    f32 = mybir.dt.float32
    bf16 = mybir.dt.float16  # fp16: same PE rate as bf16, 3 more mantissa bits

    KC = DIM // 128   # 8 k-chunks for gate/up contraction
    KH = HID // 128   # 16 k-chunks for down contraction
    NH = HID // 128   # 16 hid slices of the gate/up output
    NTOK = cpad // 128  # token tiles

    # Pair hid slices so one PSUM bank (512 fp32/partition) holds a
    # whole silu/mul group.
    PAIR = max(1, min(NH, 512 // cpad))
    W = PAIR * 128        # hid cols per chunk == silu group width
    NG = HID // W         # hid groups / weight chunks per matrix
    NDC = DIM // 512      # down-proj output column halves
    NKG = 4               # dw k-chunk groups per dc piece
    KGS = KH // NKG

    nc = bacc.Bacc("TRN2", target_bir_lowering=False, debug=False)
    # All weight tensors are pre-packed on host so that each chunk DMA
    # reads one contiguous 4KB run per partition.
    xt_d = nc.dram_tensor("xt", [128, KC, cpad], bf16, kind="ExternalInput")
    gw_d = nc.dram_tensor("gw", [NG, 128, KC, W], bf16, kind="ExternalInput")
    uw_d = nc.dram_tensor("uw", [NG, 128, KC, W], bf16, kind="ExternalInput")
    dw_d = nc.dram_tensor("dw", [NDC, NKG, 128, KGS, 512], bf16, kind="ExternalInput")
    o_d = nc.dram_tensor("o", [cpad, DIM], bf16, kind="ExternalOutput")

    with tile.TileContext(nc) as tc:
        with (
            tc.tile_pool(name="sb", bufs=1) as sb,
            tc.tile_pool(name="stmp", bufs=2) as stmp_pool,
            tc.tile_pool(name="ht", bufs=NG) as ht_pool,
            tc.tile_pool(name="outp", bufs=2) as out_pool,
            tc.tile_pool(name="psW", bufs=1, space="PSUM") as psW,
            tc.tile_pool(name="psA", bufs=2, space="PSUM") as psA,
            tc.tile_pool(name="psB", bufs=2, space="PSUM") as psB,
            tc.tile_pool(name="psO", bufs=3, space="PSUM") as psO,
        ):
            xt_s = sb.tile([128, KC, cpad], bf16)
            gw_s = sb.tile([128, NG, KC, W], bf16)
            uw_s = sb.tile([128, NG, KC, W], bf16)
            dw_s = sb.tile([128, NDC, NKG, KGS, 512], bf16)
            wu = sb.tile([128, 128], bf16)

            # PE warm-up: the HAM clock gate keeps the PE at 1.2 GHz
            # until ~3.4us of sustained activity, and an idle gap resets
            # the window. Spin dummy matmuls on a zeroed tile until the
            # first real operands (x + gate chunk 0) have landed, so real
            # GEMMs run at 2.4 GHz from the start.
            nc.vector.memset(wu[:], 0.0)
            pw = psW.tile([128, 128], f32)
            for i in range(30):
                nc.tensor.matmul(pw[:], wu[:], wu[:], start=True, stop=True,
                                 skip_group_check=True)

            # DMAs, all on the sync HWDGE ring in exact consumption
            # order (the scalar ring is kept free for output pieces).
            # x and the first gate chunk are k-split so the PE's first
            # j-slice can start after only 0.75MB has landed. Every
            # transfer is 128 fat descriptors thanks to the host-side
            # repack.
            KHALF = KC // 2
            nc.sync.dma_start(xt_s[:, 0:KHALF], xt_d.ap()[:, 0:KHALF])
            nc.sync.dma_start(gw_s[:, 0, 0:KHALF], gw_d.ap()[0][:, 0:KHALF])
            nc.sync.dma_start(xt_s[:, KHALF:], xt_d.ap()[:, KHALF:])
            nc.sync.dma_start(gw_s[:, 0, KHALF:], gw_d.ap()[0][:, KHALF:])
            nc.sync.dma_start(uw_s[:, 0], uw_d.ap()[0])
            for c in range(1, NG):
                nc.sync.dma_start(gw_s[:, c], gw_d.ap()[c])
                nc.sync.dma_start(uw_s[:, c], uw_d.ap()[c])
            for dc in range(NDC):
                for kg in range(NKG):
                    nc.sync.dma_start(dw_s[:, dc, kg], dw_d.ap()[dc, kg])

            # Gate/up grouped GEMMs; h produced in [hid, tok] layout,
            # PAIR hid slices per PSUM bank side by side. Silu is issued
            # right after the gate group so ACT overlaps the up MMs.
            ht = []
            for g in range(NG):
                pg = psA.tile([128, PAIR, cpad], f32, tag="pg")
                pu = psB.tile([128, PAIR, cpad], f32, tag="pu")
                for j in range(PAIR):
                    for k in range(KC):
                        nc.tensor.matmul(
                            pg[:, j, :], gw_s[:, g, k, j * 128:(j + 1) * 128],
                            xt_s[:, k, :],
                            start=(k == 0), stop=(k == KC - 1),
                            skip_group_check=True,
                        )
                stmp = stmp_pool.tile([128, PAIR, cpad], f32, tag="stmp")
                nc.scalar.activation(
                    stmp[:], pg[:], mybir.ActivationFunctionType.Silu
                )
                for j in range(PAIR):
                    for k in range(KC):
                        nc.tensor.matmul(
                            pu[:, j, :], uw_s[:, g, k, j * 128:(j + 1) * 128],
                            xt_s[:, k, :],
                            start=(k == 0), stop=(k == KC - 1),
                            skip_group_check=True,
                        )
                ht_t = ht_pool.tile([128, PAIR, cpad], bf16, tag="ht")
                nc.vector.tensor_mul(ht_t[:], stmp[:], pu[:])
                ht.append(ht_t)

            # Down projection: o[tok, dim] = h @ down, dc-outer so each
            # 512-col output piece is copied + DMA'd while the PE works
            # on the next piece (streams the output, shortens the tail).
            for dc in range(NDC):
                for tok in range(NTOK):
                    t0, t1 = tok * 128, (tok + 1) * 128
                    po = psO.tile([128, 512], f32, tag="po")
                    for kg in range(NKG):
                        for k2 in range(KGS):
                            kk = kg * KGS + k2
                            nc.tensor.matmul(
                                po[:],
                                ht[kk // PAIR][:, kk % PAIR, t0:t1],
                                dw_s[:, dc, kg, k2, :],
                                start=(kk == 0), stop=(kk == KH - 1),
                                skip_group_check=True,
                            )
                    out_s = out_pool.tile([128, 512], bf16, tag="out")
                    # Alternate copy engines so PSUM->SBUF evacuation of
                    # piece i overlaps piece i+1's matmuls.
                    if (dc * NTOK + tok) % 2 == 0:
                        nc.vector.tensor_copy(out_s[:], po[:])
                    else:
                        nc.scalar.copy(out_s[:], po[:])
                    nc.scalar.dma_start(
                        o_d[t0:t1, dc * 512:(dc + 1) * 512], out_s[:]
                    )

    nc.compile()
    return nc


def _get_nc(cpad: int):
    if cpad not in _cache:
        _cache[cpad] = _build(cpad)
    return _cache[cpad]


def _pack_weights(gate, up, down, cpad):
    """Repack one expert's fp16 weights into the chunk-major DRAM layout."""
    KC = DIM // 128
    NH = HID // 128
    KH = HID // 128
    PAIR = max(1, min(NH, 512 // cpad))
    W = PAIR * 128
    NG = HID // W
    NDC = DIM // 512
    NKG = 4
    KGS = KH // NKG
    gw = np.ascontiguousarray(
        gate.reshape(KC, 128, NG, W).transpose(2, 1, 0, 3))
    uw = np.ascontiguousarray(
        up.reshape(KC, 128, NG, W).transpose(2, 1, 0, 3))
    dw = np.ascontiguousarray(
        down.reshape(NKG, KGS, 128, NDC, 512).transpose(3, 0, 2, 1, 4))
    return gw, uw, dw


def _run_block(nc, xt_blocks, weights, collect):
    """One SPMD invocation: xt_blocks[e] is [128, KC, cpad] fp16."""
    from concourse.bass_utils import run_bass_kernel_spmd

    in_maps = []
    for e in range(E):
        gw, uw, dw = weights[e]
        in_maps.append({"xt": xt_blocks[e], "gw": gw, "uw": uw, "dw": dw})
    kwargs = {} if collect is None else dict(collect.get("run_kwargs") or {})
    res = run_bass_kernel_spmd(nc, in_maps, core_ids=list(range(N_CORES)), **kwargs)
    if collect is not None:
        collect.setdefault("results", []).append(res)
    return [res.results[e]["o"] for e in range(E)]


def kernel(x, counts, gate_proj, up_proj, down_proj, _collect=None):
    x = np.ascontiguousarray(np.asarray(x, dtype=np.float32))
    counts = np.asarray(counts, dtype=np.int32)
    gate_proj = np.asarray(gate_proj, dtype=np.float32).astype(BF16)
    up_proj = np.asarray(up_proj, dtype=np.float32).astype(BF16)
    down_proj = np.asarray(down_proj, dtype=np.float32).astype(BF16)

    T = x.shape[0]
    offs = np.concatenate([[0], np.cumsum(counts)]).astype(np.int64)
    cmax = int(counts.max()) if counts.size else 128

    n_blocks = max(1, -(-cmax // CMAX_BLOCK))
    if n_blocks == 1:
        cpad = max(128, -(-cmax // 128) * 128)
    else:
        cpad = CMAX_BLOCK

    KC = DIM // 128
    nc = _get_nc(cpad)
    weights = [
        _pack_weights(gate_proj[e], up_proj[e], down_proj[e], cpad)
        for e in range(E)
    ]

    out = np.empty((T, DIM), dtype=np.float32)  # o arrives fp16, upcast here
    for b in range(n_blocks):
        xt_blocks = []
        spans = []
        for e in range(E):
            c = int(counts[e])
            s0 = min(b * cpad, c)
            s1 = min((b + 1) * cpad, c)
            xe = x[offs[e] + s0:offs[e] + s1]
            if xe.shape[0] < cpad:
                xe = np.concatenate(
                    [xe, np.zeros((cpad - xe.shape[0], DIM), np.float32)], axis=0
                )
            # [cpad, DIM] -> [128, KC, cpad], 4KB contiguous per partition
            xt = np.ascontiguousarray(
                xe.astype(BF16).reshape(cpad, KC, 128).transpose(2, 1, 0))
            xt_blocks.append(xt)
            spans.append((s0, s1))
        outs = _run_block(nc, xt_blocks, weights, _collect)
        for e in range(E):
            s0, s1 = spans[e]
            if s1 > s0:
                out[offs[e] + s0:offs[e] + s1] = outs[e][: s1 - s0]
    return out


# revision 4
# speedup vs baseline: 1.0135x; 1.0135x over previous
"""MoE GroupedExperts kernel for 8 TRN2 NeuronCores.

Expert-parallel: expert e's tokens + weights go to core e. Tokens are
pre-sorted by expert, so routing is host-side slicing. Each core runs a
SwiGLU MLP: o = (silu(x @ gate) * (x @ up)) @ down.

Weights are repacked host-side into partition-major, chunk-major DRAM
layout so every weight DMA is 128 descriptors x 4KB contiguous runs
(vs 1024 x 256-512B in the natural [DIM, HID] layout) -- this moves
effective HBM bandwidth from ~268 GB/s to near the ~358 GB/s cap, which
is what paces this kernel. The PE is kept warm from t~7us (framework
preamble end) with throwaway matmuls so real GEMMs run at 2.4 GHz.
"""

import sys

if "/opt/trn_rl_repo" not in sys.path:
    sys.path.insert(0, "/opt/trn_rl_repo")

import numpy as np

BF16 = np.float16
E = 8
DIM = 1024
HID = 2048
N_CORES = 8
CMAX_BLOCK = 512  # max tokens per device invocation (PSUM free-dim limit)

_cache = {}


def _build(cpad: int):
    """Build + compile the per-core kernel for cpad tokens per expert."""
    from concourse import bacc
    import concourse.tile as tile
    import concourse.mybir as mybir

    f32 = mybir.dt.float32
    bf16 = mybir.dt.float16  # fp16: same PE rate as bf16, 3 more mantissa bits

    KC = DIM // 128   # 8 k-chunks for gate/up contraction
    KH = HID // 128   # 16 k-chunks for down contraction
    NH = HID // 128   # 16 hid slices of the gate/up output
    NTOK = cpad // 128  # token tiles

    # Pair hid slices so one PSUM bank (512 fp32/partition) holds a
    # whole silu/mul group.
    PAIR = max(1, min(NH, 512 // cpad))
    W = PAIR * 128        # hid cols per chunk == silu group width
    NG = HID // W         # hid groups / weight chunks per matrix
    NDC = DIM // 512      # down-proj output column halves
    NKG = 4               # dw k-chunk groups per dc piece
    KGS = KH // NKG

    nc = bacc.Bacc("TRN2", target_bir_lowering=False, debug=False)
    # All weight tensors are pre-packed on host so that each chunk DMA
    # reads one contiguous 4KB run per partition.
    xt_d = nc.dram_tensor("xt", [128, KC, cpad], bf16, kind="ExternalInput")
    gw_d = nc.dram_tensor("gw", [NG, 128, KC, W], bf16, kind="ExternalInput")
    uw_d = nc.dram_tensor("uw", [NG, 128, KC, W], bf16, kind="ExternalInput")
    dw_d = nc.dram_tensor("dw", [NDC, NKG, 128, KGS, 512], bf16, kind="ExternalInput")
    o_d = nc.dram_tensor("o", [cpad, DIM], bf16, kind="ExternalOutput")

    with tile.TileContext(nc) as tc:
        with (
            tc.tile_pool(name="sb", bufs=1) as sb,
            tc.tile_pool(name="stmp", bufs=2) as stmp_pool,
            tc.tile_pool(name="ht", bufs=NG) as ht_pool,
            tc.tile_pool(name="outp", bufs=2) as out_pool,
            tc.tile_pool(name="psW", bufs=1, space="PSUM") as psW,
            tc.tile_pool(name="psA", bufs=2, space="PSUM") as psA,
            tc.tile_pool(name="psB", bufs=2, space="PSUM") as psB,
            tc.tile_pool(name="psO", bufs=3, space="PSUM") as psO,
        ):
            xt_s = sb.tile([128, KC, cpad], bf16)
            gw_s = sb.tile([128, NG, KC, W], bf16)
            uw_s = sb.tile([128, NG, KC, W], bf16)
            dw_s = sb.tile([128, NDC, NKG, KGS, 512], bf16)
            wu = sb.tile([128, 128], bf16)

            # PE warm-up: the HAM clock gate keeps the PE at 1.2 GHz
            # until ~3.4us of sustained activity, and an idle gap resets
            # the window. Spin dummy matmuls on a zeroed tile until the
            # first real operands (x + gate chunk 0) have landed, so real
            # GEMMs run at 2.4 GHz from the start.
            nc.vector.memset(wu[:], 0.0)
            pw = psW.tile([128, 128], f32)
            # 44 x ~107ns (cold) bridges from preamble end (~7.6us) to
            # the first real operand arrival (~12.3us) with no PE gap.
            for i in range(44):
                nc.tensor.matmul(pw[:], wu[:], wu[:], start=True, stop=True,
                                 skip_group_check=True)

            # DMAs. x on the scalar HWDGE ring (drains concurrently with
            # the first weight chunk); weights on the sync ring in exact
            # consumption order. Every transfer is 128 fat descriptors
            # thanks to the host-side repack.
            nc.scalar.dma_start(xt_s[:], xt_d.ap())
            for c in range(NG):
                nc.sync.dma_start(gw_s[:, c], gw_d.ap()[c])
                nc.sync.dma_start(uw_s[:, c], uw_d.ap()[c])
            for dc in range(NDC):
                for kg in range(NKG):
                    nc.sync.dma_start(dw_s[:, dc, kg], dw_d.ap()[dc, kg])

            # Gate/up grouped GEMMs; h produced in [hid, tok] layout,
            # PAIR hid slices per PSUM bank side by side. Silu is issued
            # right after the gate group so ACT overlaps the up MMs.
            ht = []
            for g in range(NG):
                pg = psA.tile([128, PAIR, cpad], f32, tag="pg")
                pu = psB.tile([128, PAIR, cpad], f32, tag="pu")
                for j in range(PAIR):
                    for k in range(KC):
                        nc.tensor.matmul(
                            pg[:, j, :], gw_s[:, g, k, j * 128:(j + 1) * 128],
                            xt_s[:, k, :],
                            start=(k == 0), stop=(k == KC - 1),
                            skip_group_check=True,
                        )
                stmp = stmp_pool.tile([128, PAIR, cpad], f32, tag="stmp")
                nc.scalar.activation(
                    stmp[:], pg[:], mybir.ActivationFunctionType.Silu
                )
                for j in range(PAIR):
                    for k in range(KC):
                        nc.tensor.matmul(
                            pu[:, j, :], uw_s[:, g, k, j * 128:(j + 1) * 128],
                            xt_s[:, k, :],
                            start=(k == 0), stop=(k == KC - 1),
                            skip_group_check=True,
                        )
                ht_t = ht_pool.tile([128, PAIR, cpad], bf16, tag="ht")
                nc.vector.tensor_mul(ht_t[:], stmp[:], pu[:])
                ht.append(ht_t)

            # Down projection: o[tok, dim] = h @ down, dc-outer so each
            # 512-col output piece is copied + DMA'd while the PE works
            # on the next piece (streams the output, shortens the tail).
            for dc in range(NDC):
                for tok in range(NTOK):
                    t0, t1 = tok * 128, (tok + 1) * 128
                    po = psO.tile([128, 512], f32, tag="po")
                    for kg in range(NKG):
                        for k2 in range(KGS):
                            kk = kg * KGS + k2
                            nc.tensor.matmul(
                                po[:],
                                ht[kk // PAIR][:, kk % PAIR, t0:t1],
                                dw_s[:, dc, kg, k2, :],
                                start=(kk == 0), stop=(kk == KH - 1),
                                skip_group_check=True,
                            )
                    out_s = out_pool.tile([128, 512], bf16, tag="out")
                    # Alternate copy engines so PSUM->SBUF evacuation of
                    # piece i overlaps piece i+1's matmuls.
                    if (dc * NTOK + tok) % 2 == 0:
                        nc.vector.tensor_copy(out_s[:], po[:])
                    else:
                        nc.scalar.copy(out_s[:], po[:])
                    nc.scalar.dma_start(
                        o_d[t0:t1, dc * 512:(dc + 1) * 512], out_s[:]
                    )

    nc.compile()
    return nc


def _get_nc(cpad: int):
    if cpad not in _cache:
        _cache[cpad] = _build(cpad)
    return _cache[cpad]


def _pack_weights(gate, up, down, cpad):
    """Repack one expert's fp16 weights into the chunk-major DRAM layout."""
    KC = DIM // 128
    NH = HID // 128
    KH = HID // 128
    PAIR = max(1, min(NH, 512 // cpad))
    W = PAIR * 128
    NG = HID // W
    NDC = DIM // 512
    NKG = 4
    KGS = KH // NKG
    gw = np.ascontiguousarray(
        gate.reshape(KC, 128, NG, W).transpose(2, 1, 0, 3))
    uw = np.ascontiguousarray(
        up.reshape(KC, 128, NG, W).transpose(2, 1, 0, 3))
    dw = np.ascontiguousarray(
        down.reshape(NKG, KGS, 128, NDC, 512).transpose(3, 0, 2, 1, 4))
    return gw, uw, dw


def _run_block(nc, xt_blocks, weights, collect):
    """One SPMD invocation: xt_blocks[e] is [128, KC, cpad] fp16."""
    from concourse.bass_utils import run_bass_kernel_spmd

    in_maps = []
    for e in range(E):
        gw, uw, dw = weights[e]
        in_maps.append({"xt": xt_blocks[e], "gw": gw, "uw": uw, "dw": dw})
    kwargs = {} if collect is None else dict(collect.get("run_kwargs") or {})
    res = run_bass_kernel_spmd(nc, in_maps, core_ids=list(range(N_CORES)), **kwargs)
    if collect is not None:
        collect.setdefault("results", []).append(res)
    return [res.results[e]["o"] for e in range(E)]


def kernel(x, counts, gate_proj, up_proj, down_proj, _collect=None):
    x = np.ascontiguousarray(np.asarray(x, dtype=np.float32))
    counts = np.asarray(counts, dtype=np.int32)
    gate_proj = np.asarray(gate_proj, dtype=np.float32).astype(BF16)
    up_proj = np.asarray(up_proj, dtype=np.float32).astype(BF16)
    down_proj = np.asarray(down_proj, dtype=np.float32).astype(BF16)

    T = x.shape[0]
    offs = np.concatenate([[0], np.cumsum(counts)]).astype(np.int64)
    cmax = int(counts.max()) if counts.size else 128

    n_blocks = max(1, -(-cmax // CMAX_BLOCK))
    if n_blocks == 1:
        cpad = max(128, -(-cmax // 128) * 128)
    else:
        cpad = CMAX_BLOCK

    KC = DIM // 128
    nc = _get_nc(cpad)
    weights = [
        _pack_weights(gate_proj[e], up_proj[e], down_proj[e], cpad)
        for e in range(E)
    ]

    out = np.empty((T, DIM), dtype=np.float32)  # o arrives fp16, upcast here
    for b in range(n_blocks):
        xt_blocks = []
        spans = []
        for e in range(E):
            c = int(counts[e])
            s0 = min(b * cpad, c)
            s1 = min((b + 1) * cpad, c)
            xe = x[offs[e] + s0:offs[e] + s1]
            if xe.shape[0] < cpad:
                xe = np.concatenate(
                    [xe, np.zeros((cpad - xe.shape[0], DIM), np.float32)], axis=0
                )
            # [cpad, DIM] -> [128, KC, cpad], 4KB contiguous per partition
            xt = np.ascontiguousarray(
                xe.astype(BF16).reshape(cpad, KC, 128).transpose(2, 1, 0))
            xt_blocks.append(xt)
            spans.append((s0, s1))
        outs = _run_block(nc, xt_blocks, weights, _collect)
        for e in range(E):
            s0, s1 = spans[e]
            if s1 > s0:
                out[offs[e] + s0:offs[e] + s1] = outs[e][: s1 - s0]
    return out


# revision 5
# speedup vs baseline: 1.0480x; 1.0340x over previous
"""MoE GroupedExperts kernel for 8 TRN2 NeuronCores.

Expert-parallel: expert e's tokens + weights go to core e. Tokens are
pre-sorted by expert, so routing is host-side slicing. Each core runs a
SwiGLU MLP: o = (silu(x @ gate) * (x @ up)) @ down.

Weights are repacked host-side into partition-major, chunk-major DRAM
layout so every weight DMA is 128 descriptors x 4KB contiguous runs
(vs 1024 x 256-512B in the natural [DIM, HID] layout) -- this moves
effective HBM bandwidth from ~268 GB/s to near the ~358 GB/s cap, which
is what paces this kernel. The PE is kept warm from t~7us (framework
preamble end) with throwaway matmuls so real GEMMs run at 2.4 GHz.
"""

import sys

if "/opt/trn_rl_repo" not in sys.path:
    sys.path.insert(0, "/opt/trn_rl_repo")

import numpy as np

BF16 = np.float16
E = 8
DIM = 1024
HID = 2048
N_CORES = 8
CMAX_BLOCK = 512  # max tokens per device invocation (PSUM free-dim limit)

_cache = {}


def _build(cpad: int):
    """Build + compile the per-core kernel for cpad tokens per expert."""
    from concourse import bacc
    import concourse.tile as tile
    import concourse.mybir as mybir

    f32 = mybir.dt.float32
    bf16 = mybir.dt.float16  # fp16: same PE rate as bf16, 3 more mantissa bits

    KC = DIM // 128   # 8 k-chunks for gate/up contraction
    KH = HID // 128   # 16 k-chunks for down contraction
    NH = HID // 128   # 16 hid slices of the gate/up output
    NTOK = cpad // 128  # token tiles

    # Pair hid slices so one PSUM bank (512 fp32/partition) holds a
    # whole silu/mul group.
    PAIR = max(1, min(NH, 512 // cpad))
    W = PAIR * 128        # hid cols per chunk == silu group width
    NG = HID // W         # hid groups / weight chunks per matrix
    NDC = DIM // 512      # down-proj output column halves
    NKG = 4               # dw k-chunk groups per dc piece
    KGS = KH // NKG

    nc = bacc.Bacc("TRN2", target_bir_lowering=False, debug=False)
    # All weight tensors are pre-packed on host so that each chunk DMA
    # reads one contiguous 4KB run per partition.
    xt_d = nc.dram_tensor("xt", [128, KC, cpad], bf16, kind="ExternalInput")
    gw_d = nc.dram_tensor("gw", [NG, 128, KC, W], bf16, kind="ExternalInput")
    uw_d = nc.dram_tensor("uw", [NG, 128, KC, W], bf16, kind="ExternalInput")
    dw_d = nc.dram_tensor("dw", [NDC, NKG, 128, KGS, 512], bf16, kind="ExternalInput")
    o_d = nc.dram_tensor("o", [cpad, DIM], bf16, kind="ExternalOutput")

    with tile.TileContext(nc) as tc:
        with (
            tc.tile_pool(name="sb", bufs=1) as sb,
            tc.tile_pool(name="stmp", bufs=2) as stmp_pool,
            tc.tile_pool(name="ht", bufs=NG) as ht_pool,
            tc.tile_pool(name="outp", bufs=2) as out_pool,
            tc.tile_pool(name="psW", bufs=1, space="PSUM") as psW,
            tc.tile_pool(name="psA", bufs=2, space="PSUM") as psA,
            tc.tile_pool(name="psB", bufs=2, space="PSUM") as psB,
            tc.tile_pool(name="psO", bufs=3, space="PSUM") as psO,
        ):
            xt_s = sb.tile([128, KC, cpad], bf16)
            gw_s = sb.tile([128, NG, KC, W], bf16)
            uw_s = sb.tile([128, NG, KC, W], bf16)
            dw_s = sb.tile([128, NDC, NKG, KGS, 512], bf16)
            wu = sb.tile([128, 128], bf16)

            # PE warm-up: the HAM clock gate keeps the PE at 1.2 GHz
            # until ~3.4us of sustained activity, and an idle gap resets
            # the window. Spin dummy matmuls on a zeroed tile until the
            # first real operands (x + gate chunk 0) have landed, so real
            # GEMMs run at 2.4 GHz from the start.
            nc.vector.memset(wu[:], 0.0)
            pw = psW.tile([128, 128], f32)
            # 44 x ~107ns (cold) bridges from preamble end (~7.6us) to
            # the first real operand arrival (~12.3us) with no PE gap.
            for i in range(44):
                nc.tensor.matmul(pw[:], wu[:], wu[:], start=True, stop=True,
                                 skip_group_check=True)

            # DMAs. x on the scalar HWDGE ring (drains concurrently with
            # the first weight chunk); weights on the sync ring in exact
            # consumption order. Every transfer is 128 fat descriptors
            # thanks to the host-side repack.
            nc.scalar.dma_start(xt_s[:], xt_d.ap())
            for c in range(NG):
                nc.sync.dma_start(gw_s[:, c], gw_d.ap()[c])
                nc.sync.dma_start(uw_s[:, c], uw_d.ap()[c])
            for dc in range(NDC):
                for kg in range(NKG):
                    nc.sync.dma_start(dw_s[:, dc, kg], dw_d.ap()[dc, kg])

            # Gate/up grouped GEMMs; h produced in [hid, tok] layout,
            # PAIR hid slices per PSUM bank side by side. Silu is issued
            # right after the gate group so ACT overlaps the up MMs.
            ht = []
            for g in range(NG):
                pg = psA.tile([128, PAIR, cpad], f32, tag="pg")
                pu = psB.tile([128, PAIR, cpad], f32, tag="pu")
                for j in range(PAIR):
                    for k in range(KC):
                        nc.tensor.matmul(
                            pg[:, j, :], gw_s[:, g, k, j * 128:(j + 1) * 128],
                            xt_s[:, k, :],
                            start=(k == 0), stop=(k == KC - 1),
                            skip_group_check=True,
                        )
                stmp = stmp_pool.tile([128, PAIR, cpad], f32, tag="stmp")
                nc.scalar.activation(
                    stmp[:], pg[:], mybir.ActivationFunctionType.Silu
                )
                for j in range(PAIR):
                    for k in range(KC):
                        nc.tensor.matmul(
                            pu[:, j, :], uw_s[:, g, k, j * 128:(j + 1) * 128],
                            xt_s[:, k, :],
                            start=(k == 0), stop=(k == KC - 1),
                            skip_group_check=True,
                        )
                ht_t = ht_pool.tile([128, PAIR, cpad], bf16, tag="ht")
                nc.vector.tensor_mul(ht_t[:], stmp[:], pu[:])
                ht.append(ht_t)

            # Down projection: o[tok, dim] = h @ down, dc-outer so each
            # 512-col output piece is copied + DMA'd while the PE works
            # on the next piece (streams the output, shortens the tail).
            for dc in range(NDC):
                for tok in range(NTOK):
                    t0, t1 = tok * 128, (tok + 1) * 128
                    last = (dc == NDC - 1 and tok == NTOK - 1)
                    po = psO.tile([128, 512], f32, tag="po")
                    # The final piece is computed in two 256-col halves
                    # so the first half's copy + DMA (and its completion
                    # receipt) overlap the second half's matmuls --
                    # shortens the end-of-kernel tail by ~1us.
                    halves = ((0, 256), (256, 512)) if last else ((0, 512),)
                    for h0, h1 in halves:
                        for kg in range(NKG):
                            for k2 in range(KGS):
                                kk = kg * KGS + k2
                                nc.tensor.matmul(
                                    po[:, h0:h1],
                                    ht[kk // PAIR][:, kk % PAIR, t0:t1],
                                    dw_s[:, dc, kg, k2, h0:h1],
                                    start=(kk == 0), stop=(kk == KH - 1),
                                    skip_group_check=True,
                                )
                        out_s = out_pool.tile([128, h1 - h0], bf16, tag="out")
                        # Alternate copy engines so PSUM->SBUF evacuation
                        # of piece i overlaps piece i+1's matmuls.
                        if (dc * NTOK + tok + (h0 > 0)) % 2 == 0:
                            nc.vector.tensor_copy(out_s[:], po[:, h0:h1])
                        else:
                            nc.scalar.copy(out_s[:], po[:, h0:h1])
                        nc.scalar.dma_start(
                            o_d[t0:t1, dc * 512 + h0:dc * 512 + h1], out_s[:]
                        )

    nc.compile()
    return nc


def _get_nc(cpad: int):
    if cpad not in _cache:
        _cache[cpad] = _build(cpad)
    return _cache[cpad]


def _pack_weights(gate, up, down, cpad):
    """Repack one expert's fp16 weights into the chunk-major DRAM layout."""
    KC = DIM // 128
    NH = HID // 128
    KH = HID // 128
    PAIR = max(1, min(NH, 512 // cpad))
    W = PAIR * 128
    NG = HID // W
    NDC = DIM // 512
    NKG = 4
    KGS = KH // NKG
    gw = np.ascontiguousarray(
        gate.reshape(KC, 128, NG, W).transpose(2, 1, 0, 3))
    uw = np.ascontiguousarray(
        up.reshape(KC, 128, NG, W).transpose(2, 1, 0, 3))
    dw = np.ascontiguousarray(
        down.reshape(NKG, KGS, 128, NDC, 512).transpose(3, 0, 2, 1, 4))
    return gw, uw, dw


def _run_block(nc, xt_blocks, weights, collect):
    """One SPMD invocation: xt_blocks[e] is [128, KC, cpad] fp16."""
    from concourse.bass_utils import run_bass_kernel_spmd

    in_maps = []
    for e in range(E):
        gw, uw, dw = weights[e]
        in_maps.append({"xt": xt_blocks[e], "gw": gw, "uw": uw, "dw": dw})
    kwargs = {} if collect is None else dict(collect.get("run_kwargs") or {})
    res = run_bass_kernel_spmd(nc, in_maps, core_ids=list(range(N_CORES)), **kwargs)
    if collect is not None:
        collect.setdefault("results", []).append(res)
    return [res.results[e]["o"] for e in range(E)]


def kernel(x, counts, gate_proj, up_proj, down_proj, _collect=None):
    x = np.ascontiguousarray(np.asarray(x, dtype=np.float32))
    counts = np.asarray(counts, dtype=np.int32)
    gate_proj = np.asarray(gate_proj, dtype=np.float32).astype(BF16)
    up_proj = np.asarray(up_proj, dtype=np.float32).astype(BF16)
    down_proj = np.asarray(down_proj, dtype=np.float32).astype(BF16)

    T = x.shape[0]
    offs = np.concatenate([[0], np.cumsum(counts)]).astype(np.int64)
    cmax = int(counts.max()) if counts.size else 128

    n_blocks = max(1, -(-cmax // CMAX_BLOCK))
    if n_blocks == 1:
        cpad = max(128, -(-cmax // 128) * 128)
    else:
        cpad = CMAX_BLOCK

    KC = DIM // 128
    nc = _get_nc(cpad)
    weights = [
        _pack_weights(gate_proj[e], up_proj[e], down_proj[e], cpad)
        for e in range(E)
    ]

    out = np.empty((T, DIM), dtype=np.float32)  # o arrives fp16, upcast here
    for b in range(n_blocks):
        xt_blocks = []
        spans = []
        for e in range(E):
            c = int(counts[e])
            s0 = min(b * cpad, c)
            s1 = min((b + 1) * cpad, c)
            xe = x[offs[e] + s0:offs[e] + s1]
            if xe.shape[0] < cpad:
                xe = np.concatenate(
                    [xe, np.zeros((cpad - xe.shape[0], DIM), np.float32)], axis=0
                )
            # [cpad, DIM] -> [128, KC, cpad], 4KB contiguous per partition
            xt = np.ascontiguousarray(
                xe.astype(BF16).reshape(cpad, KC, 128).transpose(2, 1, 0))
            xt_blocks.append(xt)
            spans.append((s0, s1))
        outs = _run_block(nc, xt_blocks, weights, _collect)
        for e in range(E):
            s0, s1 = spans[e]
            if s1 > s0:
                out[offs[e] + s0:offs[e] + s1] = outs[e][: s1 - s0]
    return out


# revision 7
# speedup vs baseline: 1.0602x; 1.0117x over previous
"""MoE GroupedExperts kernel for 8 TRN2 NeuronCores.

Expert-parallel: expert e's tokens + weights go to core e. Tokens are
pre-sorted by expert, so routing is host-side slicing. Each core runs a
SwiGLU MLP: o = (silu(x @ gate) * (x @ up)) @ down.

Weights are repacked host-side into partition-major, chunk-major DRAM
layout so every weight DMA is 128 descriptors x 4KB contiguous runs
(vs 1024 x 256-512B in the natural [DIM, HID] layout) -- this moves
effective HBM bandwidth from ~268 GB/s to near the ~358 GB/s cap, which
is what paces this kernel. The PE is kept warm from t~7us (framework
preamble end) with throwaway matmuls so real GEMMs run at 2.4 GHz.
"""

import sys

if "/opt/trn_rl_repo" not in sys.path:
    sys.path.insert(0, "/opt/trn_rl_repo")

import numpy as np

BF16 = np.float16
E = 8
DIM = 1024
HID = 2048
N_CORES = 8
CMAX_BLOCK = 512  # max tokens per device invocation (PSUM free-dim limit)

_cache = {}


def _build(cpad: int):
    """Build + compile the per-core kernel for cpad tokens per expert."""
    from concourse import bacc
    import concourse.tile as tile
    import concourse.mybir as mybir

    f32 = mybir.dt.float32
    bf16 = mybir.dt.float16  # fp16: same PE rate as bf16, 3 more mantissa bits

    KC = DIM // 128   # 8 k-chunks for gate/up contraction
    KH = HID // 128   # 16 k-chunks for down contraction
    NH = HID // 128   # 16 hid slices of the gate/up output
    NTOK = cpad // 128  # token tiles

    # Pair hid slices so one PSUM bank (512 fp32/partition) holds a
    # whole silu/mul group.
    PAIR = max(1, min(NH, 512 // cpad))
    W = PAIR * 128        # hid cols per chunk == silu group width
    NG = HID // W         # hid groups / weight chunks per matrix
    NDC = DIM // 512      # down-proj output column halves
    NKG = 4               # dw k-chunk groups per dc piece
    KGS = KH // NKG

    nc = bacc.Bacc("TRN2", target_bir_lowering=False, debug=False)
    # All weight tensors are pre-packed on host so that each chunk DMA
    # reads one contiguous 4KB run per partition.
    xt_d = nc.dram_tensor("xt", [128, KC, cpad], bf16, kind="ExternalInput")
    gw_d = nc.dram_tensor("gw", [NG, 128, KC, W], bf16, kind="ExternalInput")
    uw_d = nc.dram_tensor("uw", [NG, 128, KC, W], bf16, kind="ExternalInput")
    dw_d = nc.dram_tensor("dw", [NDC, NKG, 128, KGS, 512], bf16, kind="ExternalInput")
    o_d = nc.dram_tensor("o", [cpad, DIM], bf16, kind="ExternalOutput")

    with tile.TileContext(nc) as tc:
        with (
            tc.tile_pool(name="sb", bufs=1) as sb,
            tc.tile_pool(name="stmp", bufs=2) as stmp_pool,
            tc.tile_pool(name="ht", bufs=NG) as ht_pool,
            tc.tile_pool(name="outp", bufs=2) as out_pool,
            tc.tile_pool(name="psW", bufs=1, space="PSUM") as psW,
            tc.tile_pool(name="psA", bufs=2, space="PSUM") as psA,
            tc.tile_pool(name="psB", bufs=2, space="PSUM") as psB,
            tc.tile_pool(name="psO", bufs=3, space="PSUM") as psO,
        ):
            xt_s = sb.tile([128, KC, cpad], bf16)
            gw_s = sb.tile([128, NG, KC, W], bf16)
            uw_s = sb.tile([128, NG, KC, W], bf16)
            dw_s = sb.tile([128, NDC, NKG, KGS, 512], bf16)
            wu = sb.tile([128, 128], bf16)

            # PE warm-up: the HAM clock gate keeps the PE at 1.2 GHz
            # until ~3.4us of sustained activity, and an idle gap resets
            # the window. Spin dummy matmuls on a zeroed tile until the
            # first real operands (x + gate chunk 0) have landed, so real
            # GEMMs run at 2.4 GHz from the start.
            nc.vector.memset(wu[:], 0.0)
            pw = psW.tile([128, 128], f32)
            # 44 x ~107ns (cold) bridges from preamble end (~7.6us) to
            # the first real operand arrival (~12.3us) with no PE gap.
            for i in range(44):
                nc.tensor.matmul(pw[:], wu[:], wu[:], start=True, stop=True,
                                 skip_group_check=True)

            # DMAs. x on the scalar HWDGE ring (drains concurrently with
            # the first weight chunk); weights on the sync ring in exact
            # consumption order. Every transfer is 128 fat descriptors
            # thanks to the host-side repack.
            nc.scalar.dma_start(xt_s[:], xt_d.ap())
            for c in range(NG):
                nc.sync.dma_start(gw_s[:, c], gw_d.ap()[c])
                nc.sync.dma_start(uw_s[:, c], uw_d.ap()[c])
            for dc in range(NDC):
                for kg in range(NKG):
                    nc.sync.dma_start(dw_s[:, dc, kg], dw_d.ap()[dc, kg])

            # Gate/up grouped GEMMs; h produced in [hid, tok] layout,
            # PAIR hid slices per PSUM bank side by side. Silu is issued
            # right after the gate group so ACT overlaps the up MMs.
            ht = []
            for g in range(NG):
                pg = psA.tile([128, PAIR, cpad], f32, tag="pg")
                pu = psB.tile([128, PAIR, cpad], f32, tag="pu")
                for j in range(PAIR):
                    for k in range(KC):
                        nc.tensor.matmul(
                            pg[:, j, :], gw_s[:, g, k, j * 128:(j + 1) * 128],
                            xt_s[:, k, :],
                            start=(k == 0), stop=(k == KC - 1),
                            skip_group_check=True,
                        )
                stmp = stmp_pool.tile([128, PAIR, cpad], f32, tag="stmp")
                nc.scalar.activation(
                    stmp[:], pg[:], mybir.ActivationFunctionType.Silu
                )
                for j in range(PAIR):
                    for k in range(KC):
                        nc.tensor.matmul(
                            pu[:, j, :], uw_s[:, g, k, j * 128:(j + 1) * 128],
                            xt_s[:, k, :],
                            start=(k == 0), stop=(k == KC - 1),
                            skip_group_check=True,
                        )
                ht_t = ht_pool.tile([128, PAIR, cpad], bf16, tag="ht")
                nc.vector.tensor_mul(ht_t[:], stmp[:], pu[:])
                ht.append(ht_t)

            # Down projection: o[tok, dim] = h @ down, dc-outer so each
            # 512-col output piece is copied + DMA'd while the PE works
            # on the next piece (streams the output, shortens the tail).
            for dc in range(NDC):
                for tok in range(NTOK):
                    t0, t1 = tok * 128, (tok + 1) * 128
                    last = (dc == NDC - 1 and tok == NTOK - 1)
                    # The final piece is computed in two 256-col halves
                    # (each in its OWN psum tile, so half-b's matmuls
                    # don't serialize behind half-a's copy) -- the first
                    # half's copy + DMA + completion receipt overlap the
                    # second half's matmuls, shortening the kernel tail.
                    halves = ((0, 256), (256, 512)) if last else ((0, 512),)
                    for h0, h1 in halves:
                        po = psO.tile([128, h1 - h0], f32, tag="po")
                        for kg in range(NKG):
                            for k2 in range(KGS):
                                kk = kg * KGS + k2
                                nc.tensor.matmul(
                                    po[:],
                                    ht[kk // PAIR][:, kk % PAIR, t0:t1],
                                    dw_s[:, dc, kg, k2, h0:h1],
                                    start=(kk == 0), stop=(kk == KH - 1),
                                    skip_group_check=True,
                                )
                        out_s = out_pool.tile([128, h1 - h0], bf16, tag="out")
                        # Alternate copy engines so PSUM->SBUF evacuation
                        # of piece i overlaps piece i+1's matmuls.
                        if (dc * NTOK + tok + (h0 > 0)) % 2 == 0:
                            nc.vector.tensor_copy(out_s[:], po[:])
                        else:
                            nc.scalar.copy(out_s[:], po[:])
                        nc.scalar.dma_start(
                            o_d[t0:t1, dc * 512 + h0:dc * 512 + h1], out_s[:]
                        )

    nc.compile()
    return nc


def _get_nc(cpad: int):
    if cpad not in _cache:
        _cache[cpad] = _build(cpad)
    return _cache[cpad]


def _pack_weights(gate, up, down, cpad):
    """Repack one expert's fp16 weights into the chunk-major DRAM layout."""
    KC = DIM // 128
    NH = HID // 128
    KH = HID // 128
    PAIR = max(1, min(NH, 512 // cpad))
    W = PAIR * 128
    NG = HID // W
    NDC = DIM // 512
    NKG = 4
    KGS = KH // NKG
    gw = np.ascontiguousarray(
        gate.reshape(KC, 128, NG, W).transpose(2, 1, 0, 3))
    uw = np.ascontiguousarray(
        up.reshape(KC, 128, NG, W).transpose(2, 1, 0, 3))
    dw = np.ascontiguousarray(
        down.reshape(NKG, KGS, 128, NDC, 512).transpose(3, 0, 2, 1, 4))
    return gw, uw, dw


def _run_block(nc, xt_blocks, weights, collect):
    """One SPMD invocation: xt_blocks[e] is [128, KC, cpad] fp16."""
    from concourse.bass_utils import run_bass_kernel_spmd

    in_maps = []
    for e in range(E):
        gw, uw, dw = weights[e]
        in_maps.append({"xt": xt_blocks[e], "gw": gw, "uw": uw, "dw": dw})
    kwargs = {} if collect is None else dict(collect.get("run_kwargs") or {})
    res = run_bass_kernel_spmd(nc, in_maps, core_ids=list(range(N_CORES)), **kwargs)
    if collect is not None:
        collect.setdefault("results", []).append(res)
    return [res.results[e]["o"] for e in range(E)]


def kernel(x, counts, gate_proj, up_proj, down_proj, _collect=None):
    x = np.ascontiguousarray(np.asarray(x, dtype=np.float32))
    counts = np.asarray(counts, dtype=np.int32)
    gate_proj = np.asarray(gate_proj, dtype=np.float32).astype(BF16)
    up_proj = np.asarray(up_proj, dtype=np.float32).astype(BF16)
    down_proj = np.asarray(down_proj, dtype=np.float32).astype(BF16)

    T = x.shape[0]
    offs = np.concatenate([[0], np.cumsum(counts)]).astype(np.int64)
    cmax = int(counts.max()) if counts.size else 128

    n_blocks = max(1, -(-cmax // CMAX_BLOCK))
    if n_blocks == 1:
        cpad = max(128, -(-cmax // 128) * 128)
    else:
        cpad = CMAX_BLOCK

    KC = DIM // 128
    nc = _get_nc(cpad)
    weights = [
        _pack_weights(gate_proj[e], up_proj[e], down_proj[e], cpad)
        for e in range(E)
    ]

    out = np.empty((T, DIM), dtype=np.float32)  # o arrives fp16, upcast here
    for b in range(n_blocks):
        xt_blocks = []
        spans = []
        for e in range(E):
            c = int(counts[e])
            s0 = min(b * cpad, c)
            s1 = min((b + 1) * cpad, c)
            xe = x[offs[e] + s0:offs[e] + s1]
            if xe.shape[0] < cpad:
                xe = np.concatenate(
                    [xe, np.zeros((cpad - xe.shape[0], DIM), np.float32)], axis=0
                )
            # [cpad, DIM] -> [128, KC, cpad], 4KB contiguous per partition
            xt = np.ascontiguousarray(
                xe.astype(BF16).reshape(cpad, KC, 128).transpose(2, 1, 0))
            xt_blocks.append(xt)
            spans.append((s0, s1))
        outs = _run_block(nc, xt_blocks, weights, _collect)
        for e in range(E):
            s0, s1 = spans[e]
            if s1 > s0:
                out[offs[e] + s0:offs[e] + s1] = outs[e][: s1 - s0]
    return out
